# revision 30
# baseline (speedup 1.0000x reference)
"""Distributed FlashRotarySelfAttention kernel for 8 TRN2 NeuronCores.

Reference computation (per nn_FlashRotarySelfAttention):
  qkv = x @ Wqkv;  k, q, v = split(qkv, 3)  [k first!]
  k, q = rope(k), rope(q)
  out = causal_softmax(q k^T / sqrt(Dh)) @ v
  return out @ Wproj

Sharding: tensor-parallel over heads for QKV+attention, position-parallel
for the projection. Core i owns heads {2i, 2i+1}:
  - column-parallel Wqkv (k|q|v columns of its 2 heads)
  - attention fully local per (batch, head)
  - two 8-core AllToAlls per batch (one per position half) redistribute
    attention outputs from head-sharding to position-sharding; they
    overlap trailing attention compute and the projection
  - projection: each core multiplies its 256 positions by the FULL
    (resident) Wproj producing s-major output rows (no host transpose)

All inputs are pre-cast/pre-transposed to bf16 on the host: x arrives
c-major tiled [128, 16, 4096] so QKV needs no on-chip cast or transpose.
Matmuls run bf16 with fp32 PSUM accumulation.

Softmax: causal masking is an additive -30000 bias written into the score
PSUM by an identity-weight matmul (start=True) before the score matmul
accumulates onto it; exp runs on the Scalar engine over PAIRS of k-tiles
(one [128,1024] PSUM read) to halve engine handoffs; the denominator is
accumulated across k-tiles by an all-ones matmul into a second PSUM bank
(broadcast across partitions); normalization is reciprocal_approx_fast +
one multiply on DVE. The two head-groups of each q-chunk are interleaved
so one group's exp hides behind the other's matmuls. RoPE uses
sign-folded cos/sin tables: 2 half-width + 2 full-width DVE ops per
128-channel group.

Queues: gpsimd = weight loads + collectives; scalar = xt streaming + ACT
ops; sync = tables, attention-out scatter, gather/output DMA.
"""

from contextlib import ExitStack

import numpy as np
import ml_dtypes

import concourse.bacc as bacc
import concourse.mybir as mybir
import concourse.tile as tile
from concourse.bass_utils import run_bass_kernel_spmd

# Problem shapes (hardcoded per contest rules).
B, S, C, H = 2, 2048, 2048, 16
Dh = C // H                      # 128
BS = B * S                       # 4096
N_CORES = 8
H_LOC = H // N_CORES             # 2 heads per core
W_LOC = 3 * H_LOC * Dh           # 768 local qkv columns
POS_LOC = S // N_CORES           # 256 positions per core per batch
ROPE_THETA = 10000.0
SCALE = float(Dh) ** -0.5
NEG_BIAS = -30000.0              # additive causal-mask bias (pre-scale)

F32 = mybir.dt.float32
BF16 = mybir.dt.bfloat16

P = 128            # partitions
QCH = 512          # q-chunk (matmul free dim)
N_CC = C // P      # 16 contraction chunks
N_QC = S // QCH    # 4 q-chunks per batch
N_KT = S // P      # 16 k-tiles per batch
HPOS = POS_LOC // 2  # 128 positions per AllToAll half


def _host_constants():
    """Input-independent tables computed on host (compile-time constants)."""
    half = Dh // 2
    inv_freq = 1.0 / (ROPE_THETA ** (np.arange(0, half, dtype=np.float64) / half))
    ang = np.arange(S, dtype=np.float64)[None, :] * inv_freq[:, None]   # [64, S]
    cos = np.cos(ang)
    sin = np.sin(ang)
    # Full-width rope tables: out = t*cosf + swap_halves(t)*sinn
    cosf = np.concatenate([cos, cos], axis=0).astype(ml_dtypes.bfloat16)
    sinn = np.concatenate([-sin, sin], axis=0).astype(ml_dtypes.bfloat16)
    # Causal bias triangle for the diagonal 128x128 block of each score
    # tile: keep (bias 0) iff q_local >= k_local, else NEG_BIAS.
    kk = np.arange(P)[:, None]
    qq = np.arange(P)[None, :]
    tri = np.where(qq >= kk, 0.0, NEG_BIAS).astype(ml_dtypes.bfloat16)
    ident = np.eye(P, dtype=ml_dtypes.bfloat16)
    ones = np.ones((P, P), dtype=ml_dtypes.bfloat16)
    return cosf, sinn, tri, ident, ones


def _tile_cmajor(a):
    """[C, N] -> [128, C//128, N] with channel c -> (c % 128, c // 128)."""
    cdim, n = a.shape
    return np.ascontiguousarray(
        a.reshape(cdim // P, P, n).transpose(1, 0, 2)
    )


def build_nc():
    nc = bacc.Bacc(None, num_devices=N_CORES)

    xt_in = nc.declare_dram_parameter("xt", [P, BS // QCH, N_CC, QCH], BF16, isOutput=False)
    wqkv_in = nc.declare_dram_parameter("wqkv", [P, N_CC, W_LOC], BF16, isOutput=False)
    wproj_in = nc.declare_dram_parameter("wproj", [P, N_CC, C], BF16, isOutput=False)
    cosf_in = nc.declare_dram_parameter("cosf", [Dh, S], BF16, isOutput=False)
    sinn_in = nc.declare_dram_parameter("sinn", [Dh, S], BF16, isOutput=False)
    tri_in = nc.declare_dram_parameter("tri", [P, P], BF16, isOutput=False)
    ident_in = nc.declare_dram_parameter("ident", [P, P], BF16, isOutput=False)
    ones_in = nc.declare_dram_parameter("ones", [P, P], BF16, isOutput=False)
    out_ext = nc.declare_dram_parameter("out", [B * POS_LOC, C], F32, isOutput=True)

    with tile.TileContext(nc) as tc, ExitStack() as ctx:
        consts = ctx.enter_context(tc.tile_pool(name="consts", bufs=1))
        qkvp = ctx.enter_context(tc.tile_pool(name="qkvp", bufs=1))
        xt_pool = ctx.enter_context(tc.tile_pool(name="xt", bufs=2))
        rope_pool = ctx.enter_context(tc.tile_pool(name="rope", bufs=3))
        probs_pool = ctx.enter_context(tc.tile_pool(name="probs", bufs=2))
        rec_pool = ctx.enter_context(tc.tile_pool(name="rec", bufs=1))
        accp_pool = ctx.enter_context(tc.tile_pool(name="accp", bufs=4))
        at_pool = ctx.enter_context(tc.tile_pool(name="at", bufs=2))
        gt_pool = ctx.enter_context(tc.tile_pool(name="gt", bufs=2))
        pof_pool = ctx.enter_context(tc.tile_pool(name="pof", bufs=2))
        outp_pool = ctx.enter_context(tc.tile_pool(name="outp", bufs=2))
        dram = ctx.enter_context(tc.tile_pool(name="dram", bufs=1, space="DRAM"))
        sps_pool = ctx.enter_context(tc.tile_pool(name="sps", bufs=3, space="PSUM"))
        avp_pool = ctx.enter_context(tc.tile_pool(name="avp", bufs=2, space="PSUM"))

        # ---- Startup: wqkv via sync, wproj via gpsimd (needed late) -----
        wqkv_sb = consts.tile([P, N_CC, W_LOC], BF16)
        nc.sync.dma_start(wqkv_sb[:, 0:4, :], wqkv_in[:, 0:4, :])
        nc.sync.dma_start(wqkv_sb[:, 4:16, :], wqkv_in[:, 4:16, :])
        wproj_sb = consts.tile([P, N_CC, C], BF16)

        def load_wproj():
            # scalar-queue emission point gates this 8.4MB load behind
            # chunk-3 compute so it can't steal startup DMA bandwidth
            nc.scalar.dma_start(wproj_sb[:, 0:8, :], wproj_in[:, 0:8, :])
            nc.scalar.dma_start(wproj_sb[:, 8:16, :], wproj_in[:, 8:16, :])

        cosf_sb = consts.tile([Dh, S], BF16)
        nc.sync.dma_start(cosf_sb[:], cosf_in[:])
        sinn_sb = consts.tile([Dh, S], BF16)
        nc.sync.dma_start(sinn_sb[:], sinn_in[:])
        tri_sb = consts.tile([P, P], BF16)
        nc.sync.dma_start(tri_sb[:], tri_in[:])
        ident_sb = consts.tile([P, P], BF16)
        nc.sync.dma_start(ident_sb[:], ident_in[:])
        ones_sb = consts.tile([P, P], BF16)
        nc.sync.dma_start(ones_sb[:], ones_in[:])

        # Resident activations: d-major q/k (dim1 = hl*2 + b), k-major v.
        q_sb = qkvp.tile([P, 2 * H_LOC, S], BF16)
        k_sb = qkvp.tile([P, 2 * H_LOC, S], BF16)
        v_sb = qkvp.tile([P, B, N_KT, H_LOC * Dh], BF16)

        # ---- QKV: straight bf16 loads, matmuls, full-width RoPE ---------
        def qkv_chunk(sc):
            g0 = sc * QCH
            b = g0 // S
            s0 = g0 - b * S              # position offset within batch
            xt = xt_pool.tile([P, N_CC, QCH], BF16, tag="xt", name=f"xt{sc}")
            for g in range(4):
                nc.scalar.dma_start(
                    xt[:, 4 * g:4 * (g + 1), :],
                    xt_in[:, sc, 4 * g:4 * (g + 1), :],
                )

            # k (ct 0,1) and q (ct 2,3): d-major matmul + RoPE
            cos_c = cosf_sb[:, s0:s0 + QCH]
            sin_c = sinn_sb[:, s0:s0 + QCH]
            for ct in range(4):
                ps = sps_pool.tile([P, 2 * QCH], F32, tag="sc")
                for cc in range(N_CC):
                    nc.tensor.matmul(
                        ps[:, :QCH],
                        lhsT=wqkv_sb[:, cc, ct * P:(ct + 1) * P],
                        rhs=xt[:, cc, :],
                        start=(cc == 0),
                        stop=(cc == N_CC - 1),
                    )
                hl = ct % 2
                dst = k_sb if ct < 2 else q_sb
                bh = hl * 2 + b
                # out = t*cosf + swap_halves(t)*sinn  (sign folded in sinn);
                # psum is drained to bf16 on the idle Scalar engine so the
                # DVE ops run at 16-bit (2x) rate
                tb = rope_pool.tile([P, QCH], BF16, tag="rt", name="tb")
                nc.scalar.copy(tb[:], ps[:, 0:QCH])
                tmp = rope_pool.tile([P, QCH], BF16, tag="rt")
                c1 = rope_pool.tile([P, QCH], BF16, tag="rt")
                nc.vector.tensor_tensor(tmp[0:64, :], ps[64:128, 0:QCH],
                                        sin_c[0:64, :], mybir.AluOpType.mult)
                nc.vector.tensor_tensor(tmp[64:128, :], ps[0:64, 0:QCH],
                                        sin_c[64:128, :], mybir.AluOpType.mult)
                nc.vector.tensor_tensor(c1[:], tb[:], cos_c,
                                        mybir.AluOpType.mult)
                nc.vector.tensor_tensor(
                    dst[:, bh, s0:s0 + QCH], c1[:], tmp[:],
                    mybir.AluOpType.add,
                )

            # v: s-major [pos_tile, 2 heads * Dh]
            for blk in range(QCH // P):
                st = s0 // P + blk
                pv = sps_pool.tile([P, 2 * QCH], F32, tag="sc")
                for cc in range(N_CC):
                    nc.tensor.matmul(
                        pv[:, :H_LOC * Dh],
                        lhsT=xt[:, cc, blk * P:(blk + 1) * P],
                        rhs=wqkv_sb[:, cc, 4 * P:],
                        start=(cc == 0),
                        stop=(cc == N_CC - 1),
                    )
                nc.scalar.copy(v_sb[:, b, st, :], pv[:, :H_LOC * Dh])


        # ---- Attention + chunked AllToAll + projection ------------------
        # a2a_in[b][h]: [2048, 128] = 8 shards (128-pos blocks) x 256 ch;
        # row = block*256 + hl*128 + hd, col = position within block
        a2a_in = [[dram.tile([C, HPOS], BF16, name=f"a2i{j}{h}")
                   for h in range(2)] for j in range(B)]
        a2a_out = [[dram.tile([C, HPOS], BF16, name=f"a2o{j}{h}")
                    for h in range(2)] for j in range(B)]

        def attn_qc(b, qc):
            """Both head-groups of one q-chunk, kt-pair interleaved."""
            n_kt = (QCH // P) * (qc + 1)
            po = [avp_pool.tile([P, QCH], F32, tag="po", name=f"po{i}")
                  for i in range(2)]
            acc = [accp_pool.tile([P, QCH], BF16, tag="acc", name=f"acc{i}")
                   for i in range(2)]
            for pp in range(n_kt // 2):
                kt0 = 2 * pp
                prs = [None, None]
                offs = {}
                for hl in range(2):
                    bh = hl * 2 + b
                    psc = sps_pool.tile([P, 2 * QCH], F32, tag="sc")
                    o0 = 0
                    for half in range(2):
                        kt = kt0 + half
                        jj = kt - (QCH // P) * qc
                        off = P * jj if jj > 0 else 0
                        offs[(hl, half)] = off
                        if half == 0:
                            o0 = off
                        base = half * QCH
                        if jj >= 0:
                            nc.tensor.matmul(
                                psc[:, base + P * jj:base + P * (jj + 1)],
                                lhsT=ident_sb[:], rhs=tri_sb[:],
                                start=True, stop=False,
                            )
                        nc.tensor.matmul(
                            psc[:, base + off:base + QCH],
                            lhsT=k_sb[:, bh, kt * P:(kt + 1) * P],
                            rhs=q_sb[:, bh, qc * QCH + off:(qc + 1) * QCH],
                            start=(jj < 0), stop=True,
                        )
                    pr = probs_pool.tile([P, 2 * QCH], BF16, tag="pr")
                    prs[hl] = pr
                    nc.scalar.activation(
                        pr[:, o0:], psc[:, o0:],
                        mybir.ActivationFunctionType.Exp,
                        scale=SCALE,
                    )
                for hl in range(2):
                    pr = prs[hl]
                    for half in range(2):
                        kt = kt0 + half
                        off = offs[(hl, half)]
                        base = half * QCH
                        nc.tensor.matmul(
                            po[hl][:, off:],
                            lhsT=v_sb[:, b, kt, hl * Dh:(hl + 1) * Dh],
                            rhs=pr[:, base + off:base + QCH],
                            start=(kt == 0), stop=(kt == n_kt - 1),
                        )
                        # DVE accumulation of the softmax denominator terms
                        if kt == 0:
                            nc.vector.tensor_copy(acc[hl][:], pr[:, 0:QCH])
                        else:
                            nc.vector.tensor_tensor(
                                acc[hl][:, off:], acc[hl][:, off:],
                                pr[:, base + off:base + QCH],
                                mybir.AluOpType.add,
                            )
            for hl in range(2):
                # drain po to SBUF on the Scalar engine so its PSUM bank
                # frees immediately (the next group's av-matmuls reuse it)
                pof = pof_pool.tile([P, QCH], F32, tag="pof")
                nc.scalar.copy(pof[:], po[hl][:])
                pdt = avp_pool.tile([P, QCH], F32, tag="po", name="pdt")
                nc.tensor.matmul(
                    pdt[:], lhsT=ones_sb[:], rhs=acc[hl][:],
                    start=True, stop=True,
                )
                rec = rec_pool.tile([P, QCH], F32, tag="rec")
                nc.vector.reciprocal_approx_fast(rec[:], pdt[:])
                at = at_pool.tile([P, QCH], BF16, tag="at")
                nc.vector.tensor_tensor(
                    at[:], pof[:], rec[:], mybir.AluOpType.mult
                )
                # scatter into the AllToAll input for position-half qc//2:
                # the 512 at-columns are 4 blocks of 128 positions landing
                # at shard rows ((qc%2)*4 + jl)*256 + hl*128
                for jl in range(4):
                    r0 = (4 * (qc % 2) + jl) * 2 * P + hl * P
                    nc.sync.dma_start(
                        a2a_in[b][qc // 2][r0:r0 + P, :],
                        at[:, jl * P:(jl + 1) * P],
                    )

        gts = {}

        def alltoall(b, h):
            nc.gpsimd.collective_compute(
                "AllToAll",
                mybir.AluOpType.bypass,
                replica_groups=[list(range(N_CORES))],
                ins=[a2a_in[b][h][:].opt()],
                outs=[a2a_out[b][h][:].opt()],
            )
            gt = gt_pool.tile([P, N_CC, HPOS], BF16, tag="gt", name=f"gt{b}{h}")
            nc.gpsimd.dma_start(
                gt[:], a2a_out[b][h].rearrange("(o p) q -> p o q", p=P)
            )
            gts[(b, h)] = gt

        def projection(b, h):
            gt = gts[(b, h)]
            for oc in range(C // QCH):
                ps = sps_pool.tile([P, 2 * QCH], F32, tag="sc")
                for cc in range(N_CC):
                    nc.tensor.matmul(
                        ps[:, :QCH],
                        lhsT=gt[:, cc, :],
                        rhs=wproj_sb[:, cc, oc * QCH:(oc + 1) * QCH],
                        start=(cc == 0),
                        stop=(cc == N_CC - 1),
                    )
                ot = outp_pool.tile([P, QCH], F32, tag="ot")
                nc.scalar.copy(ot[:], ps[:, :QCH])
                r0 = b * POS_LOC + h * P
                nc.sync.dma_start(
                    out_ext[r0:r0 + P, oc * QCH:(oc + 1) * QCH],
                    ot[:],
                )

        qkv_chunk(0)
        qkv_chunk(1)
        attn_qc(0, 0)
        qkv_chunk(2)
        attn_qc(0, 1)
        alltoall(0, 0)
        qkv_chunk(3)
        attn_qc(0, 2)
        load_wproj()
        qkv_chunk(4)
        attn_qc(0, 3)
        alltoall(0, 1)
        qkv_chunk(5)
        attn_qc(1, 0)
        qkv_chunk(6)
        attn_qc(1, 1)
        alltoall(1, 0)
        qkv_chunk(7)
        attn_qc(1, 2)
        attn_qc(1, 3)
        alltoall(1, 1)
        projection(0, 0)
        projection(0, 1)
        projection(1, 0)
        projection(1, 1)

    nc.finalize()
    return nc


_NC_CACHE = None


def _get_nc():
    global _NC_CACHE
    if _NC_CACHE is None:
        _NC_CACHE = build_nc()
    return _NC_CACHE


def make_in_maps(x, Wqkv, Wproj):
    """Shard + pre-transpose the full inputs across the 8 cores (host)."""
    x2 = np.asarray(x, dtype=np.float32).reshape(BS, C)
    xt = np.ascontiguousarray(
        x2.T.astype(ml_dtypes.bfloat16)
        .reshape(N_CC, P, BS // QCH, QCH).transpose(1, 2, 0, 3)
    )                                                          # [128,8,16,512]
    Wqkv = np.asarray(Wqkv, dtype=np.float32)
    Wproj = np.asarray(Wproj, dtype=np.float32)
    wproj_t = _tile_cmajor(Wproj.astype(ml_dtypes.bfloat16))  # [128,16,2048]
    cosf, sinn, tri, ident, ones = _host_constants()
    in_maps = []
    for i in range(N_CORES):
        h0 = H_LOC * i
        cols = []
        for part in range(3):  # k, q, v blocks (k first per reference)
            base = part * C + h0 * Dh
            cols.append(Wqkv[:, base:base + H_LOC * Dh])
        wqkv_loc = _tile_cmajor(
            np.concatenate(cols, axis=1).astype(ml_dtypes.bfloat16)
        )
        in_maps.append({
            "xt": xt,
            "wqkv": wqkv_loc,
            "wproj": wproj_t,
            "cosf": cosf,
            "sinn": sinn,
            "tri": tri,
            "ident": ident,
            "ones": ones,
        })
    return in_maps


def assemble_output(results):
    # core i, half h holds batch positions [h*1024 + i*128, h*1024 + (i+1)*128)
    out = np.empty((B, S, C), dtype=np.float32)
    for i in range(N_CORES):
        o = results[i]["out"].reshape(B, 2, P, C)
        for h in range(2):
            p0 = h * (S // 2) + i * P
            out[:, p0:p0 + P, :] = o[:, h]
    return out


def kernel(x, Wqkv, Wproj):
    nc = _get_nc()
    in_maps = make_in_maps(x, Wqkv, Wproj)
    res = run_bass_kernel_spmd(nc, in_maps, core_ids=list(range(N_CORES)))
    return assemble_output(res.results)


# revision 31
# speedup vs baseline: 1.0137x; 1.0137x over previous
"""Distributed FlashRotarySelfAttention kernel for 8 TRN2 NeuronCores.

Reference computation (per nn_FlashRotarySelfAttention):
  qkv = x @ Wqkv;  k, q, v = split(qkv, 3)  [k first!]
  k, q = rope(k), rope(q)
  out = causal_softmax(q k^T / sqrt(Dh)) @ v
  return out @ Wproj

Sharding: tensor-parallel over heads for QKV+attention, position-parallel
for the projection. Core i owns heads {2i, 2i+1}:
  - column-parallel Wqkv (k|q|v columns of its 2 heads)
  - attention fully local per (batch, head)
  - two 8-core AllToAlls per batch (one per position half) redistribute
    attention outputs from head-sharding to position-sharding; they
    overlap trailing attention compute and the projection
  - projection: each core multiplies its 256 positions by the FULL
    (resident) Wproj producing s-major output rows (no host transpose)

All inputs are pre-cast/pre-transposed to bf16 on the host: x arrives
c-major tiled [128, 16, 4096] so QKV needs no on-chip cast or transpose.
Matmuls run bf16 with fp32 PSUM accumulation.

Softmax: causal masking is an additive -30000 bias written into the score
PSUM by an identity-weight matmul (start=True) before the score matmul
accumulates onto it; exp runs on the Scalar engine over PAIRS of k-tiles
(one [128,1024] PSUM read) to halve engine handoffs; the denominator is
accumulated across k-tiles by an all-ones matmul into a second PSUM bank
(broadcast across partitions); normalization is reciprocal_approx_fast +
one multiply on DVE. The two head-groups of each q-chunk are interleaved
so one group's exp hides behind the other's matmuls. RoPE uses
sign-folded cos/sin tables: 2 half-width + 2 full-width DVE ops per
128-channel group.

Queues: gpsimd = weight loads + collectives; scalar = xt streaming + ACT
ops; sync = tables, attention-out scatter, gather/output DMA.
"""

from contextlib import ExitStack

import numpy as np
import ml_dtypes

import concourse.bacc as bacc
import concourse.mybir as mybir
import concourse.tile as tile
from concourse.bass_utils import run_bass_kernel_spmd

# Problem shapes (hardcoded per contest rules).
B, S, C, H = 2, 2048, 2048, 16
Dh = C // H                      # 128
BS = B * S                       # 4096
N_CORES = 8
H_LOC = H // N_CORES             # 2 heads per core
W_LOC = 3 * H_LOC * Dh           # 768 local qkv columns
POS_LOC = S // N_CORES           # 256 positions per core per batch
ROPE_THETA = 10000.0
SCALE = float(Dh) ** -0.5
NEG_BIAS = -30000.0              # additive causal-mask bias (pre-scale)

F32 = mybir.dt.float32
BF16 = mybir.dt.bfloat16

P = 128            # partitions
QCH = 512          # q-chunk (matmul free dim)
N_CC = C // P      # 16 contraction chunks
N_QC = S // QCH    # 4 q-chunks per batch
N_KT = S // P      # 16 k-tiles per batch
HPOS = POS_LOC // 2  # 128 positions per AllToAll half


def _host_constants():
    """Input-independent tables computed on host (compile-time constants)."""
    half = Dh // 2
    inv_freq = 1.0 / (ROPE_THETA ** (np.arange(0, half, dtype=np.float64) / half))
    ang = np.arange(S, dtype=np.float64)[None, :] * inv_freq[:, None]   # [64, S]
    cos = np.cos(ang)
    sin = np.sin(ang)
    # Full-width rope tables: out = t*cosf + swap_halves(t)*sinn
    cosf = np.concatenate([cos, cos], axis=0).astype(ml_dtypes.bfloat16)
    sinn = np.concatenate([-sin, sin], axis=0).astype(ml_dtypes.bfloat16)
    # Causal bias triangle for the diagonal 128x128 block of each score
    # tile: keep (bias 0) iff q_local >= k_local, else NEG_BIAS.
    kk = np.arange(P)[:, None]
    qq = np.arange(P)[None, :]
    tri = np.where(qq >= kk, 0.0, NEG_BIAS).astype(ml_dtypes.bfloat16)
    ident = np.eye(P, dtype=ml_dtypes.bfloat16)
    ones = np.ones((P, P), dtype=ml_dtypes.bfloat16)
    return cosf, sinn, tri, ident, ones


def _tile_cmajor(a):
    """[C, N] -> [128, C//128, N] with channel c -> (c % 128, c // 128)."""
    cdim, n = a.shape
    return np.ascontiguousarray(
        a.reshape(cdim // P, P, n).transpose(1, 0, 2)
    )


def build_nc():
    nc = bacc.Bacc(None, num_devices=N_CORES)

    xt_in = nc.declare_dram_parameter("xt", [P, BS // QCH, N_CC, QCH], BF16, isOutput=False)
    wqkv_in = nc.declare_dram_parameter("wqkv", [P, N_CC, W_LOC], BF16, isOutput=False)
    wproj_in = nc.declare_dram_parameter("wproj", [P, N_CC, C], BF16, isOutput=False)
    cosf_in = nc.declare_dram_parameter("cosf", [Dh, S], BF16, isOutput=False)
    sinn_in = nc.declare_dram_parameter("sinn", [Dh, S], BF16, isOutput=False)
    tri_in = nc.declare_dram_parameter("tri", [P, P], BF16, isOutput=False)
    ident_in = nc.declare_dram_parameter("ident", [P, P], BF16, isOutput=False)
    ones_in = nc.declare_dram_parameter("ones", [P, P], BF16, isOutput=False)
    out_ext = nc.declare_dram_parameter("out", [B * POS_LOC, C], F32, isOutput=True)

    with tile.TileContext(nc) as tc, ExitStack() as ctx:
        consts = ctx.enter_context(tc.tile_pool(name="consts", bufs=1))
        qkvp = ctx.enter_context(tc.tile_pool(name="qkvp", bufs=1))
        xt_pool = ctx.enter_context(tc.tile_pool(name="xt", bufs=2))
        rope_pool = ctx.enter_context(tc.tile_pool(name="rope", bufs=3))
        probs_pool = ctx.enter_context(tc.tile_pool(name="probs", bufs=2))
        rec_pool = ctx.enter_context(tc.tile_pool(name="rec", bufs=1))
        accp_pool = ctx.enter_context(tc.tile_pool(name="accp", bufs=3))
        at_pool = ctx.enter_context(tc.tile_pool(name="at", bufs=2))
        gt_pool = ctx.enter_context(tc.tile_pool(name="gt", bufs=2))
        pof_pool = ctx.enter_context(tc.tile_pool(name="pof", bufs=2))
        outp_pool = ctx.enter_context(tc.tile_pool(name="outp", bufs=2))
        dram = ctx.enter_context(tc.tile_pool(name="dram", bufs=1, space="DRAM"))
        sps_pool = ctx.enter_context(tc.tile_pool(name="sps", bufs=3, space="PSUM"))
        avp_pool = ctx.enter_context(tc.tile_pool(name="avp", bufs=2, space="PSUM"))

        # ---- Startup: wqkv via sync, wproj via gpsimd (needed late) -----
        wqkv_sb = consts.tile([P, N_CC, W_LOC], BF16)
        nc.sync.dma_start(wqkv_sb[:, 0:4, :], wqkv_in[:, 0:4, :])
        nc.sync.dma_start(wqkv_sb[:, 4:16, :], wqkv_in[:, 4:16, :])
        wproj_sb = consts.tile([P, N_CC, C], BF16)

        def load_wproj():
            # scalar-queue emission point gates this 8.4MB load behind
            # chunk-3 compute so it can't steal startup DMA bandwidth
            nc.scalar.dma_start(wproj_sb[:, 0:8, :], wproj_in[:, 0:8, :])
            nc.scalar.dma_start(wproj_sb[:, 8:16, :], wproj_in[:, 8:16, :])

        cosf_sb = consts.tile([Dh, S], BF16)
        nc.sync.dma_start(cosf_sb[:], cosf_in[:])
        sinn_sb = consts.tile([Dh, S], BF16)
        nc.sync.dma_start(sinn_sb[:], sinn_in[:])
        tri_sb = consts.tile([P, P], BF16)
        nc.sync.dma_start(tri_sb[:], tri_in[:])
        ident_sb = consts.tile([P, P], BF16)
        nc.sync.dma_start(ident_sb[:], ident_in[:])
        ones_sb = consts.tile([P, P], BF16)
        nc.sync.dma_start(ones_sb[:], ones_in[:])

        # Resident activations: d-major q/k (dim1 = hl*2 + b), k-major v.
        q_sb = qkvp.tile([P, 2 * H_LOC, S], BF16)
        k_sb = qkvp.tile([P, 2 * H_LOC, S], BF16)
        v_sb = qkvp.tile([P, B, N_KT, H_LOC * Dh], BF16)

        # ---- QKV: straight bf16 loads, matmuls, full-width RoPE ---------
        def qkv_chunk(sc):
            g0 = sc * QCH
            b = g0 // S
            s0 = g0 - b * S              # position offset within batch
            xt = xt_pool.tile([P, N_CC, QCH], BF16, tag="xt", name=f"xt{sc}")
            for g in range(4):
                nc.scalar.dma_start(
                    xt[:, 4 * g:4 * (g + 1), :],
                    xt_in[:, sc, 4 * g:4 * (g + 1), :],
                )

            # k (ct 0,1) and q (ct 2,3): d-major matmul + RoPE
            cos_c = cosf_sb[:, s0:s0 + QCH]
            sin_c = sinn_sb[:, s0:s0 + QCH]
            for ct in range(4):
                ps = sps_pool.tile([P, 2 * QCH], F32, tag="sc")
                for cc in range(N_CC):
                    nc.tensor.matmul(
                        ps[:, :QCH],
                        lhsT=wqkv_sb[:, cc, ct * P:(ct + 1) * P],
                        rhs=xt[:, cc, :],
                        start=(cc == 0),
                        stop=(cc == N_CC - 1),
                    )
                hl = ct % 2
                dst = k_sb if ct < 2 else q_sb
                bh = hl * 2 + b
                # out = t*cosf + swap_halves(t)*sinn  (sign folded in sinn);
                # psum is drained to bf16 on the idle Scalar engine so the
                # DVE ops run at 16-bit (2x) rate
                tb = rope_pool.tile([P, QCH], BF16, tag="rt", name="tb")
                nc.scalar.copy(tb[:], ps[:, 0:QCH])
                tmp = rope_pool.tile([P, QCH], BF16, tag="rt")
                c1 = rope_pool.tile([P, QCH], BF16, tag="rt")
                nc.vector.tensor_tensor(tmp[0:64, :], ps[64:128, 0:QCH],
                                        sin_c[0:64, :], mybir.AluOpType.mult)
                nc.vector.tensor_tensor(tmp[64:128, :], ps[0:64, 0:QCH],
                                        sin_c[64:128, :], mybir.AluOpType.mult)
                nc.vector.tensor_tensor(c1[:], tb[:], cos_c,
                                        mybir.AluOpType.mult)
                nc.vector.tensor_tensor(
                    dst[:, bh, s0:s0 + QCH], c1[:], tmp[:],
                    mybir.AluOpType.add,
                )

            # v: s-major [pos_tile, 2 heads * Dh]
            for blk in range(QCH // P):
                st = s0 // P + blk
                pv = sps_pool.tile([P, 2 * QCH], F32, tag="sc")
                for cc in range(N_CC):
                    nc.tensor.matmul(
                        pv[:, :H_LOC * Dh],
                        lhsT=xt[:, cc, blk * P:(blk + 1) * P],
                        rhs=wqkv_sb[:, cc, 4 * P:],
                        start=(cc == 0),
                        stop=(cc == N_CC - 1),
                    )
                nc.scalar.copy(v_sb[:, b, st, :], pv[:, :H_LOC * Dh])


        # ---- Attention + chunked AllToAll + projection ------------------
        # a2a_in[b][h]: [2048, 128] = 8 shards (128-pos blocks) x 256 ch;
        # row = block*256 + hl*128 + hd, col = position within block
        a2a_in = [[dram.tile([C, HPOS], BF16, name=f"a2i{j}{h}")
                   for h in range(2)] for j in range(B)]
        a2a_out = [[dram.tile([C, HPOS], BF16, name=f"a2o{j}{h}")
                    for h in range(2)] for j in range(B)]

        def attn_qc(b, qc):
            """Both head-groups of one q-chunk, kt-pair interleaved."""
            n_kt = (QCH // P) * (qc + 1)
            po = [avp_pool.tile([P, QCH], F32, tag="po", name=f"po{i}")
                  for i in range(2)]
            acc = [accp_pool.tile([P, QCH], BF16, tag="acc", name=f"acc{i}")
                   for i in range(2)]
            for pp in range(n_kt // 2):
                kt0 = 2 * pp
                prs = [None, None]
                offs = {}
                for hl in range(2):
                    bh = hl * 2 + b
                    psc = sps_pool.tile([P, 2 * QCH], F32, tag="sc")
                    o0 = 0
                    for half in range(2):
                        kt = kt0 + half
                        jj = kt - (QCH // P) * qc
                        off = P * jj if jj > 0 else 0
                        offs[(hl, half)] = off
                        if half == 0:
                            o0 = off
                        base = half * QCH
                        if jj >= 0:
                            nc.tensor.matmul(
                                psc[:, base + P * jj:base + P * (jj + 1)],
                                lhsT=ident_sb[:], rhs=tri_sb[:],
                                start=True, stop=False,
                            )
                        nc.tensor.matmul(
                            psc[:, base + off:base + QCH],
                            lhsT=k_sb[:, bh, kt * P:(kt + 1) * P],
                            rhs=q_sb[:, bh, qc * QCH + off:(qc + 1) * QCH],
                            start=(jj < 0), stop=True,
                        )
                    pr = probs_pool.tile([P, 2 * QCH], BF16, tag="pr")
                    prs[hl] = pr
                    nc.scalar.activation(
                        pr[:, o0:], psc[:, o0:],
                        mybir.ActivationFunctionType.Exp,
                        scale=SCALE,
                    )
                for hl in range(2):
                    pr = prs[hl]
                    for half in range(2):
                        kt = kt0 + half
                        off = offs[(hl, half)]
                        base = half * QCH
                        nc.tensor.matmul(
                            po[hl][:, off:],
                            lhsT=v_sb[:, b, kt, hl * Dh:(hl + 1) * Dh],
                            rhs=pr[:, base + off:base + QCH],
                            start=(kt == 0), stop=(kt == n_kt - 1),
                        )
                        # DVE accumulation of the softmax denominator terms
                        if kt == 0:
                            nc.vector.tensor_copy(acc[hl][:], pr[:, 0:QCH])
                        else:
                            nc.vector.tensor_tensor(
                                acc[hl][:, off:], acc[hl][:, off:],
                                pr[:, base + off:base + QCH],
                                mybir.AluOpType.add,
                            )
            for hl in range(2):
                # drain po to SBUF on the Scalar engine so its PSUM bank
                # frees immediately (the next group's av-matmuls reuse it)
                pof = pof_pool.tile([P, QCH], F32, tag="pof")
                nc.scalar.copy(pof[:], po[hl][:])
                pdt = avp_pool.tile([P, QCH], F32, tag="po", name="pdt")
                nc.tensor.matmul(
                    pdt[:], lhsT=ones_sb[:], rhs=acc[hl][:],
                    start=True, stop=True,
                )
                rec = rec_pool.tile([P, QCH], F32, tag="rec")
                nc.vector.reciprocal_approx_fast(rec[:], pdt[:])
                at = at_pool.tile([P, QCH], BF16, tag="at")
                nc.vector.tensor_tensor(
                    at[:], pof[:], rec[:], mybir.AluOpType.mult
                )
                # scatter into the AllToAll input for position-half qc//2:
                # the 512 at-columns are 4 blocks of 128 positions landing
                # at shard rows ((qc%2)*4 + jl)*256 + hl*128
                for jl in range(4):
                    r0 = (4 * (qc % 2) + jl) * 2 * P + hl * P
                    nc.sync.dma_start(
                        a2a_in[b][qc // 2][r0:r0 + P, :],
                        at[:, jl * P:(jl + 1) * P],
                    )

        gts = {}

        def alltoall(b, h):
            nc.gpsimd.collective_compute(
                "AllToAll",
                mybir.AluOpType.bypass,
                replica_groups=[list(range(N_CORES))],
                ins=[a2a_in[b][h][:].opt()],
                outs=[a2a_out[b][h][:].opt()],
            )
            gt = gt_pool.tile([P, N_CC, HPOS], BF16, tag="gt", name=f"gt{b}{h}")
            nc.gpsimd.dma_start(
                gt[:], a2a_out[b][h].rearrange("(o p) q -> p o q", p=P)
            )
            gts[(b, h)] = gt

        def projection(b, h):
            gt = gts[(b, h)]
            for oc in range(C // QCH):
                ps = sps_pool.tile([P, 2 * QCH], F32, tag="sc")
                for cc in range(N_CC):
                    nc.tensor.matmul(
                        ps[:, :QCH],
                        lhsT=gt[:, cc, :],
                        rhs=wproj_sb[:, cc, oc * QCH:(oc + 1) * QCH],
                        start=(cc == 0),
                        stop=(cc == N_CC - 1),
                    )
                ot = outp_pool.tile([P, QCH], F32, tag="ot")
                nc.scalar.copy(ot[:], ps[:, :QCH])
                r0 = b * POS_LOC + h * P
                nc.sync.dma_start(
                    out_ext[r0:r0 + P, oc * QCH:(oc + 1) * QCH],
                    ot[:],
                )

        qkv_chunk(0)
        qkv_chunk(1)
        attn_qc(0, 0)
        qkv_chunk(2)
        attn_qc(0, 1)
        alltoall(0, 0)
        qkv_chunk(3)
        attn_qc(0, 2)
        load_wproj()
        qkv_chunk(4)
        attn_qc(0, 3)
        alltoall(0, 1)
        qkv_chunk(5)
        attn_qc(1, 0)
        qkv_chunk(6)
        attn_qc(1, 1)
        alltoall(1, 0)
        qkv_chunk(7)
        attn_qc(1, 2)
        attn_qc(1, 3)
        alltoall(1, 1)
        projection(0, 0)
        projection(0, 1)
        projection(1, 0)
        projection(1, 1)

    nc.finalize()
    return nc


_NC_CACHE = None


def _get_nc():
    global _NC_CACHE
    if _NC_CACHE is None:
        _NC_CACHE = build_nc()
    return _NC_CACHE


def make_in_maps(x, Wqkv, Wproj):
    """Shard + pre-transpose the full inputs across the 8 cores (host)."""
    x2 = np.asarray(x, dtype=np.float32).reshape(BS, C)
    xt = np.ascontiguousarray(
        x2.T.astype(ml_dtypes.bfloat16)
        .reshape(N_CC, P, BS // QCH, QCH).transpose(1, 2, 0, 3)
    )                                                          # [128,8,16,512]
    Wqkv = np.asarray(Wqkv, dtype=np.float32)
    Wproj = np.asarray(Wproj, dtype=np.float32)
    wproj_t = _tile_cmajor(Wproj.astype(ml_dtypes.bfloat16))  # [128,16,2048]
    cosf, sinn, tri, ident, ones = _host_constants()
    in_maps = []
    for i in range(N_CORES):
        h0 = H_LOC * i
        cols = []
        for part in range(3):  # k, q, v blocks (k first per reference)
            base = part * C + h0 * Dh
            cols.append(Wqkv[:, base:base + H_LOC * Dh])
        wqkv_loc = _tile_cmajor(
            np.concatenate(cols, axis=1).astype(ml_dtypes.bfloat16)
        )
        in_maps.append({
            "xt": xt,
            "wqkv": wqkv_loc,
            "wproj": wproj_t,
            "cosf": cosf,
            "sinn": sinn,
            "tri": tri,
            "ident": ident,
            "ones": ones,
        })
    return in_maps


def assemble_output(results):
    # core i, half h holds batch positions [h*1024 + i*128, h*1024 + (i+1)*128)
    out = np.empty((B, S, C), dtype=np.float32)
    for i in range(N_CORES):
        o = results[i]["out"].reshape(B, 2, P, C)
        for h in range(2):
            p0 = h * (S // 2) + i * P
            out[:, p0:p0 + P, :] = o[:, h]
    return out


def kernel(x, Wqkv, Wproj):
    nc = _get_nc()
    in_maps = make_in_maps(x, Wqkv, Wproj)
    res = run_bass_kernel_spmd(nc, in_maps, core_ids=list(range(N_CORES)))
    return assemble_output(res.results)
